# revision 35
# baseline (speedup 1.0000x reference)
"""Trainium2 Bass kernel for nn_Encoder (pre-norm attention + spiking FFN).

Sharding: 8 cores = 4 batches x 2 sequence halves, pure data parallel, no
collectives.  Each core receives the full 2048-token batch row with its own
query half permuted to the front (softmax over keys is permutation
invariant), computes attention for its 1024 query tokens against all 2048
keys, plus the FFN for those tokens, and returns a [1024, 512] slice.

LayerNorm affine params and linear biases are folded on the host:
  n = xhat*g + be  =>  n @ W + b == xhat @ (g[:,None]*W) + (be@W + b)
so the device only computes plain (x-mu)*rstd layernorms.

Math per core (m-batch row, q = first 1024 tokens of xin):
  xhat1 = LN(xin)                         (all 2048 tokens)
  qT/kT = (wq'/wk')^T xhat1^T + b^T       (f16, transposed layout)
  v     = xhat1 @ wv' + bv'               (f16, natural layout, +ones col)
  S^T   = kT_h^T q_h per head             (PSUM f32)
  P^T   = exp(S^T)                        (f16; no max subtraction - scores
                                           are O(10) so f32 exp is safe)
  ctx^T = [V_h|1]^T P^T  accumulated over key chunks  -> [65, 1024]
  att   = transpose(ctx^T) rows scaled by 1/Z (Z = ones-col sums)
  x1    = xq + att
  h1^T  = w1'^T LN(x1)^T + b1'            (f32r for spike-threshold accuracy)
  spk^T = (h1^T >= 2.0)                   (f16, exact 0/1)
  out   = x1 + spk @ w2 + b2              (b2 via K=1 ones matmul)
"""

import sys
from contextlib import ExitStack

sys.path.insert(0, "/opt/trn_rl_repo")

import numpy as np

import concourse.bass as bass
import concourse.tile as tile
from concourse import mybir
from concourse.bass_utils import run_bass_kernel_spmd
from concourse.masks import make_identity
from concourse.vector_clock import ScopedClock, VectorClock

f32 = mybir.dt.float32
f32r = mybir.dt.float32r
f16 = mybir.dt.float16
bf16 = mybir.dt.bfloat16
AF = mybir.ActivationFunctionType
ALU = mybir.AluOpType

M, S, E, H, D, F = 4, 2048, 512, 8, 64, 2048
SQ = S // 2              # query tokens per core
N_CORES = 8
EPS = 1e-5
EC = E // 128            # 4 embed chunks
FC = F // 128            # 16 ffn chunks
TK = S // 128            # 16 key-token tiles
TQ = SQ // 128           # 8 query-token tiles
VW = D + 1               # per-head Vext width (64 v cols + ones col)


# --------------------------------------------------------------------------
# Tile framework patches for this toolchain: walrus rejects >1 sem-wait per
# instruction, so (a) the TileContext exit drain is replaced with a chain of
# single-wait SP nops, and (b) a post-pass splits any remaining multi-wait
# instruction into same-engine single-wait NoOps placed immediately before it
# (engines execute in order, so the wait point is unchanged).
# --------------------------------------------------------------------------

def _split_drain_and_barrier(self, tick_clock, wait_clock):
    g = tick_clock.global_clock
    n = len(g)
    for p in range(n):
        if g[p] > 0:
            vec = [g[p] if i == p else 0 for i in range(n)]
            nop = self.nc.sync.nop(nofuse=True, hint="split_drain")
            wait_clock.add_sem_waits(nop.ins, ScopedClock({None: VectorClock(vec)}))
    self.nc.sync.drain()
    self.nc.all_engine_barrier()
    assert self.sems is not None
    popped = self.nc._tile_sem_poison_stack.pop()
    assert popped is self._sem_poison
    self.nc.clear_and_free_semaphores(list(self.sems.allocated().values()))
    self.nc.all_engine_barrier()


tile.TileContext._drain_and_barrier = _split_drain_and_barrier


def split_multiwait(nc, limit=1):
    n_split = 0
    for fn in nc.m.functions:
        for bb in fn.blocks:
            il = bb.instructions
            out = []
            for inst in il:
                si = getattr(inst, "sync_info", None)
                waits = list(si.on_wait) if si is not None and si.on_wait else []
                if len(waits) > limit:
                    keep = waits[-limit:]
                    extra = waits[:-limit]
                    for j, w in enumerate(extra):
                        nop = mybir.InstNoOp(name=f"{inst.name}-wsplit{j}")
                        nop.engine = inst.engine
                        nop.sync_info = mybir.SyncInfo(on_wait=[w], on_update=[])
                        out.append(nop)
                        n_split += 1
                    inst.sync_info = mybir.SyncInfo(
                        on_wait=keep, on_update=list(si.on_update)
                    )
                out.append(inst)
            if len(out) != len(il):
                il[:] = out
    return n_split


# --------------------------------------------------------------------------
# Device program
# --------------------------------------------------------------------------

def build_nc(split=True):
    nc = bass.Bass()

    xin = nc.declare_dram_parameter("xin", [S, E], f32, isOutput=False)
    wq_d = nc.declare_dram_parameter("wq", [EC, 128, E], f16, isOutput=False)
    wk_d = nc.declare_dram_parameter("wk", [EC, 128, E], f16, isOutput=False)
    wv_d = nc.declare_dram_parameter("wv", [EC, 128, E], f16, isOutput=False)
    bqT_d = nc.declare_dram_parameter("bqT", [128, EC], f32, isOutput=False)
    bkT_d = nc.declare_dram_parameter("bkT", [128, EC], f32, isOutput=False)
    bv_d = nc.declare_dram_parameter("bv", [E], f32, isOutput=False)
    w1_d = nc.declare_dram_parameter("w1", [EC, 128, F], f16, isOutput=False)
    b1T_d = nc.declare_dram_parameter("b1T", [128, FC], f32, isOutput=False)
    w2_d = nc.declare_dram_parameter("w2", [FC, 128, E], f16, isOutput=False)
    b2_d = nc.declare_dram_parameter("b2", [1, E], f16, isOutput=False)
    out_d = nc.declare_dram_parameter("out", [SQ, E], f32, isOutput=True)

    with tile.TileContext(nc) as tc, ExitStack() as top:
        common = top.enter_context(tc.tile_pool(name="common", bufs=1))
        stats = top.enter_context(tc.tile_pool(name="stats", bufs=4))
        outp = top.enter_context(tc.tile_pool(name="outp", bufs=3))

        ident16 = common.tile([128, 128], f16, tag="ident16")
        make_identity(nc, ident16[:])
        ident32 = common.tile([128, 128], f32, tag="ident32")
        make_identity(nc, ident32[:])
        ones1 = common.tile([1, 128], f16, tag="ones1")
        nc.vector.memset(ones1[:], 1.0)
        b2_sb = common.tile([1, E], f16, tag="b2")
        nc.sync.dma_start(b2_sb[:], b2_d[:])
        bv_rep = common.tile([128, E], f32, tag="bvrep")
        bv_ap = bv_d[:]
        nc.gpsimd.dma_start(
            out=bv_rep[:],
            in_=bass.AP(tensor=bv_ap.tensor, offset=bv_ap.offset,
                        ap=[[0, 128]] + list(bv_ap.ap)),
        )
        x1 = [common.tile([128, E], f32, tag=f"x1_{t}", name=f"x1_{t}") for t in range(TQ)]
        eps_sb = common.tile([128, 1], f32, tag="eps")
        nc.vector.memset(eps_sb[:], EPS)

        def layernorm_to(dst_f16_or_f32, src_ap, on_gpsimd=False):
            st6 = stats.tile([128, 6], f32, tag="bn6")
            nc.vector.bn_stats(st6[:], src_ap)
            mv = stats.tile([128, 2], f32, tag="mv")
            nc.vector.bn_aggr(mv[:], st6[:])
            std = stats.tile([128, 1], f32, tag="std")
            nc.scalar.activation(std[:], mv[:, 1:2], AF.Sqrt, bias=eps_sb[:])
            rstd = stats.tile([128, 1], f32, tag="rstd")
            nc.vector.reciprocal(rstd[:], std[:])
            eng = nc.gpsimd if on_gpsimd else nc.vector
            eng.tensor_scalar(
                out=dst_f16_or_f32,
                in0=src_ap,
                scalar1=mv[:, 0:1],
                scalar2=rstd[:],
                op0=ALU.subtract,
                op1=ALU.mult,
            )

        with ExitStack() as attn:
            resA = attn.enter_context(tc.tile_pool(name="resA", bufs=1))
            tempA = attn.enter_context(tc.tile_pool(name="tempA", bufs=3))
            ps_st = attn.enter_context(tc.tile_pool(name="ps_st", bufs=4, space="PSUM"))
            projsc = ExitStack()
            ps_proj = projsc.enter_context(
                tc.tile_pool(name="ps_proj", bufs=2, space="PSUM")
            )

            # ---- phase A: load x, LN1, transpose to xhatT (f16) ----
            xq = [resA.tile([128, E], f32, tag=f"xq{t}", name=f"xq{t}") for t in range(TQ)]
            xhatT = [resA.tile([128, S], f16, tag=f"xhT{k}", name=f"xhT{k}") for k in range(EC)]
            for t in range(TK):
                if t < TQ:
                    xt = xq[t]
                else:
                    xt = tempA.tile([128, E], f32, tag="xkv")
                nc.sync.dma_start(xt[:], xin[t * 128:(t + 1) * 128, :])
                xh = tempA.tile([128, E], f16, tag="xh1")
                layernorm_to(xh[:], xt[:], on_gpsimd=True)
                for k in range(EC):
                    tp = ps_proj.tile([128, 128], f16, tag="t16")
                    nc.tensor.transpose(tp[:], xh[:, k * 128:(k + 1) * 128], ident16[:])
                    nc.vector.tensor_copy(xhatT[k][:, t * 128:(t + 1) * 128], tp[:])

            # ---- phase B: projections ----
            wq_sb = [resA.tile([128, E], f16, tag=f"wq{k}", name=f"wq{k}") for k in range(EC)]
            wk_sb = [resA.tile([128, E], f16, tag=f"wk{k}", name=f"wk{k}") for k in range(EC)]
            wv_sb = [resA.tile([128, E], f16, tag=f"wv{k}", name=f"wv{k}") for k in range(EC)]
            for k in range(EC):
                nc.sync.dma_start(wq_sb[k][:], wq_d[k])
                nc.sync.dma_start(wk_sb[k][:], wk_d[k])
                nc.sync.dma_start(wv_sb[k][:], wv_d[k])
            bqT = resA.tile([128, EC], f32, tag="bqT")
            bkT = resA.tile([128, EC], f32, tag="bkT")
            nc.sync.dma_start(bqT[:], bqT_d[:])
            nc.sync.dma_start(bkT[:], bkT_d[:])

            qT = [resA.tile([128, SQ], f16, tag=f"qT{k}", name=f"qT{k}") for k in range(EC)]
            kT = [resA.tile([128, S], f16, tag=f"kT{k}", name=f"kT{k}") for k in range(EC)]
            vext = [resA.tile([128, H * VW], bf16, tag=f"vx{t}", name=f"vx{t}") for t in range(TK)]

            def project_v(t):
                ps = ps_proj.tile([128, E], f32, tag="mm512", name="vps")
                for k in range(EC):
                    nc.tensor.matmul(
                        ps[:],
                        xhatT[k][:, t * 128:(t + 1) * 128],
                        wv_sb[k][:],
                        start=(k == 0),
                        stop=(k == EC - 1),
                    )
                vv = vext[t].rearrange("p (h c) -> p h c", c=VW)
                nc.vector.tensor_add(
                    vv[:, :, 0:D],
                    ps[:].rearrange("p (h c) -> p h c", c=D),
                    bv_rep[:].rearrange("p (h c) -> p h c", c=D),
                )
                nc.gpsimd.memset(vv[:, :, D:VW], 1.0)

            for dc in range(EC):
                for h2 in range(SQ // 512):
                    ps = ps_proj.tile([128, 512], f32, tag="mm512")
                    for k in range(EC):
                        nc.tensor.matmul(
                            ps[:],
                            wq_sb[k][:, dc * 128:(dc + 1) * 128],
                            xhatT[k][:, h2 * 512:(h2 + 1) * 512],
                            start=(k == 0),
                            stop=(k == EC - 1),
                        )
                    nc.vector.tensor_scalar(
                        out=qT[dc][:, h2 * 512:(h2 + 1) * 512],
                        in0=ps[:],
                        scalar1=bqT[:, dc:dc + 1],
                        scalar2=None,
                        op0=ALU.add,
                    )
                for h2 in range(S // 512):
                    ps = ps_proj.tile([128, 512], f32, tag="mm512")
                    for k in range(EC):
                        nc.tensor.matmul(
                            ps[:],
                            wk_sb[k][:, dc * 128:(dc + 1) * 128],
                            xhatT[k][:, h2 * 512:(h2 + 1) * 512],
                            start=(k == 0),
                            stop=(k == EC - 1),
                        )
                    nc.vector.tensor_scalar(
                        out=kT[dc][:, h2 * 512:(h2 + 1) * 512],
                        in0=ps[:],
                        scalar1=bkT[:, dc:dc + 1],
                        scalar2=None,
                        op0=ALU.add,
                    )
                if dc == 0:
                    for t in range(TK):
                        project_v(t)

            projsc.close()
            ps_ctx = attn.enter_context(
                tc.tile_pool(name="ps_ctx", bufs=2, space="PSUM")
            )

            # ---- phase C: attention, head pairs ----
            att_sb = [resA.tile([128, E], f32, tag=f"att{t}", name=f"att{t}") for t in range(TQ)]
            ptodd = [resA.tile([128, SQ], bf16, tag=f"pto{kc}", name=f"pto{kc}") for kc in range(TK)]

            def head_epilogue(h, ctx_ps):
                ctxs = tempA.tile([VW, SQ], f32, tag="ctxs")
                nc.vector.tensor_copy(ctxs[:], ctx_ps[:])
                for qc in range(TQ):
                    tp = ps_st.tile([128, VW], f32, tag="st", name="tp")
                    nc.tensor.transpose(
                        tp[:], ctxs[:, qc * 128:(qc + 1) * 128], ident32[0:VW, 0:VW]
                    )
                    rec = stats.tile([128, 1], f32, tag="zrec")
                    nc.vector.reciprocal(rec[:], tp[:, D:VW])
                    nc.vector.tensor_scalar(
                        out=att_sb[qc][:, h * D:(h + 1) * D],
                        in0=tp[:, 0:D],
                        scalar1=rec[:],
                        scalar2=None,
                        op0=ALU.mult,
                    )

            for hp in range(H // 2):
                h0, h1 = 2 * hp, 2 * hp + 1
                ctx0 = ps_ctx.tile([VW, SQ], f32, tag="ctx", name="ctx0")
                for kc in range(TK):
                    pt0 = tempA.tile([128, SQ], bf16, tag="pte", name="pte")
                    pts = {h0: pt0, h1: ptodd[kc]}
                    for half in range(SQ // 512):
                        for h in (h0, h1):
                            dc, row = h // 2, (h % 2) * D
                            st = ps_st.tile([128, 512], f32, tag="st", name="st")
                            nc.tensor.matmul(
                                st[:],
                                kT[dc][row:row + D, kc * 128:(kc + 1) * 128],
                                qT[dc][row:row + D, half * 512:(half + 1) * 512],
                                start=True,
                                stop=True,
                            )
                            nc.scalar.activation(
                                pts[h][:, half * 512:(half + 1) * 512], st[:], AF.Exp
                            )
                        nc.tensor.matmul(
                            ctx0[:, half * 512:(half + 1) * 512],
                            vext[kc][:, h0 * VW:(h0 + 1) * VW],
                            pt0[:, half * 512:(half + 1) * 512],
                            start=(kc == 0),
                            stop=(kc == TK - 1),
                        )
                head_epilogue(h0, ctx0)
                ctx1 = ps_ctx.tile([VW, SQ], f32, tag="ctx", name="ctx1")
                for kc in range(TK):
                    for half in range(SQ // 512):
                        nc.tensor.matmul(
                            ctx1[:, half * 512:(half + 1) * 512],
                            vext[kc][:, h1 * VW:(h1 + 1) * VW],
                            ptodd[kc][:, half * 512:(half + 1) * 512],
                            start=(kc == 0),
                            stop=(kc == TK - 1),
                        )
                head_epilogue(h1, ctx1)

            # ---- phase D: residual ----
            for qc in range(TQ):
                nc.gpsimd.tensor_add(x1[qc][:], xq[qc][:], att_sb[qc][:])

        # ---- phase E/F/G: LN2 + FFN ----
        with ExitStack() as ffn:
            resB = ffn.enter_context(tc.tile_pool(name="resB", bufs=1))
            tempB = ffn.enter_context(tc.tile_pool(name="tempB", bufs=3))
            ps_h1 = ffn.enter_context(tc.tile_pool(name="ps_h1", bufs=2, space="PSUM"))
            ps_f2 = ffn.enter_context(tc.tile_pool(name="ps_f2", bufs=2, space="PSUM"))

            xh2T = [resB.tile([128, SQ], f32r, tag=f"x2T{k}", name=f"x2T{k}") for k in range(EC)]
            for qc in range(TQ):
                xh2 = tempB.tile([128, E], f32, tag="xh2")
                layernorm_to(xh2[:], x1[qc][:])
                for k in range(EC):
                    tp = ps_f2.tile([128, 128], f32, tag="t32")
                    nc.tensor.transpose(
                        tp[:], xh2[:, k * 128:(k + 1) * 128], ident32[:]
                    )
                    nc.scalar.copy(xh2T[k][:, qc * 128:(qc + 1) * 128], tp[:])

            w1_sb = [resB.tile([128, F], f16, tag=f"w1{k}", name=f"w1{k}") for k in range(EC)]
            for k in range(EC):
                nc.sync.dma_start(w1_sb[k][:], w1_d[k])
            b1T = resB.tile([128, FC], f32, tag="b1T")
            nc.sync.dma_start(b1T[:], b1T_d[:])
            w2_sb = [resB.tile([128, E], f16, tag=f"w2{c}", name=f"w2{c}") for c in range(FC)]
            for c in range(FC):
                nc.sync.dma_start(w2_sb[c][:], w2_d[c])

            spkT = [resB.tile([128, SQ], f16, tag=f"spk{c}", name=f"spk{c}") for c in range(FC)]
            for half in range(SQ // 512):
                for c in range(FC):
                    ps = ps_h1.tile([128, 512], f32, tag="h1")
                    for k in range(EC):
                        nc.tensor.matmul(
                            ps[:],
                            w1_sb[k][:, c * 128:(c + 1) * 128],
                            xh2T[k][:, half * 512:(half + 1) * 512],
                            start=(k == 0),
                            stop=(k == EC - 1),
                        )
                    nc.vector.tensor_scalar(
                        out=spkT[c][:, half * 512:(half + 1) * 512],
                        in0=ps[:],
                        scalar1=b1T[:, c:c + 1],
                        scalar2=2.0,
                        op0=ALU.add,
                        op1=ALU.is_ge,
                    )
                for qc in range(half * (TQ // 2), (half + 1) * (TQ // 2)):
                    ps = ps_f2.tile([128, E], f32, tag="mm512")
                    for c in range(FC):
                        nc.tensor.matmul(
                            ps[:],
                            spkT[c][:, qc * 128:(qc + 1) * 128],
                            w2_sb[c][:],
                            start=(c == 0),
                            stop=False,
                        )
                    nc.tensor.matmul(ps[:], ones1[:], b2_sb[:], start=False, stop=True)
                    ot = outp.tile([128, E], f32, tag="ot")
                    nc.vector.tensor_add(ot[:], x1[qc][:], ps[:])
                    nc.sync.dma_start(out_d[qc * 128:(qc + 1) * 128, :], ot[:])

    if split:
        split_multiwait(nc)
    return nc


_NC = None


def _get_nc():
    global _NC
    if _NC is None:
        _NC = build_nc()
    return _NC


# --------------------------------------------------------------------------
# Host wrapper
# --------------------------------------------------------------------------

def _prep_weights(inputs):
    f = lambda k: np.asarray(inputs[k], np.float32)
    g1, be1 = f("g1"), f("be1")
    g2, be2 = f("g2"), f("be2")
    wq, wk, wv = f("wq"), f("wk"), f("wv")
    bq, bk, bv = f("bq"), f("bk"), f("bv")
    w1, b1 = f("w1"), f("b1")
    w2, b2 = f("w2"), f("b2")

    wq_e = wq * g1[:, None]
    wk_e = wk * g1[:, None]
    wv_e = wv * g1[:, None]
    bq_e = bq + be1 @ wq
    bk_e = bk + be1 @ wk
    bv_e = bv + be1 @ wv
    w1_e = w1 * g2[:, None]
    b1_e = b1 + be2 @ w1

    return {
        "wq": wq_e.reshape(EC, 128, E).astype(np.float16),
        "wk": wk_e.reshape(EC, 128, E).astype(np.float16),
        "wv": wv_e.reshape(EC, 128, E).astype(np.float16),
        "bqT": np.ascontiguousarray(bq_e.reshape(EC, 128).T),
        "bkT": np.ascontiguousarray(bk_e.reshape(EC, 128).T),
        "bv": bv_e,
        "w1": np.ascontiguousarray(w1_e.reshape(EC, 128, F)).astype(np.float16),
        "b1T": np.ascontiguousarray(b1_e.reshape(FC, 128).T),
        "w2": np.ascontiguousarray(w2.reshape(FC, 128, E)).astype(np.float16),
        "b2": b2.reshape(1, E).astype(np.float16),
    }


def _run(inputs, **spmd_kwargs):
    x = np.asarray(inputs["x"], np.float32)
    w = _prep_weights(inputs)
    in_maps = []
    for c in range(N_CORES):
        b, h = c // 2, c % 2
        xq = x[b, h * SQ:(h + 1) * SQ]
        xo = x[b, (1 - h) * SQ:(2 - h) * SQ]
        m = dict(w)
        m["xin"] = np.ascontiguousarray(np.concatenate([xq, xo], axis=0))
        in_maps.append(m)
    res = run_bass_kernel_spmd(_get_nc(), in_maps, list(range(N_CORES)), **spmd_kwargs)
    out = np.empty((M, S, E), np.float32)
    for c in range(N_CORES):
        b, h = c // 2, c % 2
        out[b, h * SQ:(h + 1) * SQ] = res.results[c]["out"]
    return out, res


def kernel(**inputs):
    try:
        out, _ = _run(inputs)
    except Exception:
        # transient device hiccups (NRT exec-unit resets) recover on retry
        out, _ = _run(inputs)
    return out


# revision 37
# speedup vs baseline: 1.2391x; 1.2391x over previous
"""Trainium2 Bass kernel for nn_Encoder (pre-norm attention + spiking FFN).

Sharding: 8 cores = 4 batches x 2 sequence halves, pure data parallel, no
collectives.  Each core receives the full 2048-token batch row with its own
query half permuted to the front (softmax over keys is permutation
invariant), computes attention for its 1024 query tokens against all 2048
keys, plus the FFN for those tokens, and returns a [1024, 512] slice.

LayerNorm affine params and linear biases are folded on the host:
  n = xhat*g + be  =>  n @ W + b == xhat @ (g[:,None]*W) + (be@W + b)
so the device only computes plain (x-mu)*rstd layernorms.

Math per core (m-batch row, q = first 1024 tokens of xin):
  xhat1 = LN(xin)                         (all 2048 tokens)
  qT/kT = (wq'/wk')^T xhat1^T + b^T       (f16, transposed layout)
  v     = xhat1 @ wv' + bv'               (f16, natural layout, +ones col)
  S^T   = kT_h^T q_h per head             (PSUM f32)
  P^T   = exp(S^T)                        (f16; no max subtraction - scores
                                           are O(10) so f32 exp is safe)
  ctx^T = [V_h|1]^T P^T  accumulated over key chunks  -> [65, 1024]
  att   = transpose(ctx^T) rows scaled by 1/Z (Z = ones-col sums)
  x1    = xq + att
  h1^T  = w1'^T LN(x1)^T + b1'            (f32r for spike-threshold accuracy)
  spk^T = (h1^T >= 2.0)                   (f16, exact 0/1)
  out   = x1 + spk @ w2 + b2              (b2 via K=1 ones matmul)
"""

import sys
from contextlib import ExitStack

sys.path.insert(0, "/opt/trn_rl_repo")

import numpy as np

import concourse.bass as bass
import concourse.tile as tile
from concourse import mybir
from concourse.bass_utils import run_bass_kernel_spmd
from concourse.masks import make_identity
from concourse.vector_clock import ScopedClock, VectorClock

f32 = mybir.dt.float32
f32r = mybir.dt.float32r
f16 = mybir.dt.float16
bf16 = mybir.dt.bfloat16
AF = mybir.ActivationFunctionType
ALU = mybir.AluOpType

M, S, E, H, D, F = 4, 2048, 512, 8, 64, 2048
SQ = S // 2              # query tokens per core
N_CORES = 8
EPS = 1e-5
EC = E // 128            # 4 embed chunks
FC = F // 128            # 16 ffn chunks
TK = S // 128            # 16 key-token tiles
TQ = SQ // 128           # 8 query-token tiles
VW = D + 1               # per-head Vext width (64 v cols + ones col)


# --------------------------------------------------------------------------
# Tile framework patches for this toolchain: walrus rejects >1 sem-wait per
# instruction, so (a) the TileContext exit drain is replaced with a chain of
# single-wait SP nops, and (b) a post-pass splits any remaining multi-wait
# instruction into same-engine single-wait NoOps placed immediately before it
# (engines execute in order, so the wait point is unchanged).
# --------------------------------------------------------------------------

def _split_drain_and_barrier(self, tick_clock, wait_clock):
    g = tick_clock.global_clock
    n = len(g)
    for p in range(n):
        if g[p] > 0:
            vec = [g[p] if i == p else 0 for i in range(n)]
            nop = self.nc.sync.nop(nofuse=True, hint="split_drain")
            wait_clock.add_sem_waits(nop.ins, ScopedClock({None: VectorClock(vec)}))
    self.nc.sync.drain()
    self.nc.all_engine_barrier()
    assert self.sems is not None
    popped = self.nc._tile_sem_poison_stack.pop()
    assert popped is self._sem_poison
    self.nc.clear_and_free_semaphores(list(self.sems.allocated().values()))
    self.nc.all_engine_barrier()


tile.TileContext._drain_and_barrier = _split_drain_and_barrier


def split_multiwait(nc, limit=1):
    n_split = 0
    for fn in nc.m.functions:
        for bb in fn.blocks:
            il = bb.instructions
            out = []
            for inst in il:
                si = getattr(inst, "sync_info", None)
                waits = list(si.on_wait) if si is not None and si.on_wait else []
                if len(waits) > limit:
                    keep = waits[-limit:]
                    extra = waits[:-limit]
                    for j, w in enumerate(extra):
                        nop = mybir.InstNoOp(name=f"{inst.name}-wsplit{j}")
                        nop.engine = inst.engine
                        nop.sync_info = mybir.SyncInfo(on_wait=[w], on_update=[])
                        out.append(nop)
                        n_split += 1
                    inst.sync_info = mybir.SyncInfo(
                        on_wait=keep, on_update=list(si.on_update)
                    )
                out.append(inst)
            if len(out) != len(il):
                il[:] = out
    return n_split


# --------------------------------------------------------------------------
# Device program
# --------------------------------------------------------------------------

def build_nc(split=True):
    nc = bass.Bass()

    xin = nc.declare_dram_parameter("xin", [S, E], f32, isOutput=False)
    wq_d = nc.declare_dram_parameter("wq", [EC, 128, E], f16, isOutput=False)
    wk_d = nc.declare_dram_parameter("wk", [EC, 128, E], f16, isOutput=False)
    wv_d = nc.declare_dram_parameter("wv", [EC, 128, E], f16, isOutput=False)
    bqT_d = nc.declare_dram_parameter("bqT", [128, EC], f32, isOutput=False)
    bkT_d = nc.declare_dram_parameter("bkT", [128, EC], f32, isOutput=False)
    bv_d = nc.declare_dram_parameter("bv", [E], f32, isOutput=False)
    w1_d = nc.declare_dram_parameter("w1", [EC, 128, F], f32r, isOutput=False)
    b1T_d = nc.declare_dram_parameter("b1T", [128, FC], f32, isOutput=False)
    w2_d = nc.declare_dram_parameter("w2", [FC, 128, E], f16, isOutput=False)
    b2_d = nc.declare_dram_parameter("b2", [1, E], f16, isOutput=False)
    out_d = nc.declare_dram_parameter("out", [SQ, E], f32, isOutput=True)

    with tile.TileContext(nc) as tc, ExitStack() as top:
        common = top.enter_context(tc.tile_pool(name="common", bufs=1))
        stats = top.enter_context(tc.tile_pool(name="stats", bufs=6))
        outp = top.enter_context(tc.tile_pool(name="outp", bufs=4))

        ident16 = common.tile([128, 128], f16, tag="ident16")
        make_identity(nc, ident16[:])
        ident32 = common.tile([128, 128], f32, tag="ident32")
        make_identity(nc, ident32[:])
        ones1 = common.tile([1, 128], f16, tag="ones1")
        nc.vector.memset(ones1[:], 1.0)
        b2_sb = common.tile([1, E], f16, tag="b2")
        nc.sync.dma_start(b2_sb[:], b2_d[:])
        bv_rep = common.tile([128, E], f32, tag="bvrep")
        bv_ap = bv_d[:]
        nc.gpsimd.dma_start(
            out=bv_rep[:],
            in_=bass.AP(tensor=bv_ap.tensor, offset=bv_ap.offset,
                        ap=[[0, 128]] + list(bv_ap.ap)),
        )
        x1 = [common.tile([128, E], f32, tag=f"x1_{t}", name=f"x1_{t}") for t in range(TQ)]
        eps_sb = common.tile([128, 1], f32, tag="eps")
        nc.vector.memset(eps_sb[:], EPS)

        def layernorm_to(dst_f16_or_f32, src_ap):
            st6 = stats.tile([128, 6], f32, tag="bn6")
            nc.vector.bn_stats(st6[:], src_ap)
            mv = stats.tile([128, 2], f32, tag="mv")
            nc.vector.bn_aggr(mv[:], st6[:])
            std = stats.tile([128, 1], f32, tag="std")
            nc.scalar.activation(std[:], mv[:, 1:2], AF.Sqrt, bias=eps_sb[:])
            rstd = stats.tile([128, 1], f32, tag="rstd")
            nc.vector.reciprocal(rstd[:], std[:])
            nc.vector.tensor_scalar(
                out=dst_f16_or_f32,
                in0=src_ap,
                scalar1=mv[:, 0:1],
                scalar2=rstd[:],
                op0=ALU.subtract,
                op1=ALU.mult,
            )

        with ExitStack() as attn:
            resA = attn.enter_context(tc.tile_pool(name="resA", bufs=1))
            tempA = attn.enter_context(tc.tile_pool(name="tempA", bufs=4))
            ps_st = attn.enter_context(tc.tile_pool(name="ps_st", bufs=4, space="PSUM"))
            projsc = ExitStack()
            ps_proj = projsc.enter_context(
                tc.tile_pool(name="ps_proj", bufs=2, space="PSUM")
            )

            # ---- phase A: load x, LN1, transpose to xhatT (f16) ----
            xq = [resA.tile([128, E], f32, tag=f"xq{t}", name=f"xq{t}") for t in range(TQ)]
            xhatT = [resA.tile([128, S], f16, tag=f"xhT{k}", name=f"xhT{k}") for k in range(EC)]
            for t in range(TK):
                if t < TQ:
                    xt = xq[t]
                else:
                    xt = tempA.tile([128, E], f32, tag="xkv")
                nc.sync.dma_start(xt[:], xin[t * 128:(t + 1) * 128, :])
                xh = tempA.tile([128, E], f16, tag="xh1")
                layernorm_to(xh[:], xt[:])
                for k in range(EC):
                    tp = ps_proj.tile([128, 128], f16, tag="t16")
                    nc.tensor.transpose(tp[:], xh[:, k * 128:(k + 1) * 128], ident16[:])
                    nc.vector.tensor_copy(xhatT[k][:, t * 128:(t + 1) * 128], tp[:])

            # ---- phase B: projections ----
            wq_sb = [resA.tile([128, E], f16, tag=f"wq{k}", name=f"wq{k}") for k in range(EC)]
            wk_sb = [resA.tile([128, E], f16, tag=f"wk{k}", name=f"wk{k}") for k in range(EC)]
            wv_sb = [resA.tile([128, E], f16, tag=f"wv{k}", name=f"wv{k}") for k in range(EC)]
            for k in range(EC):
                nc.sync.dma_start(wq_sb[k][:], wq_d[k])
                nc.sync.dma_start(wk_sb[k][:], wk_d[k])
                nc.sync.dma_start(wv_sb[k][:], wv_d[k])
            bqT = resA.tile([128, EC], f32, tag="bqT")
            bkT = resA.tile([128, EC], f32, tag="bkT")
            nc.sync.dma_start(bqT[:], bqT_d[:])
            nc.sync.dma_start(bkT[:], bkT_d[:])

            qT = [resA.tile([128, SQ], f16, tag=f"qT{k}", name=f"qT{k}") for k in range(EC)]
            kT = [resA.tile([128, S], f16, tag=f"kT{k}", name=f"kT{k}") for k in range(EC)]
            vext = [resA.tile([128, H * VW], bf16, tag=f"vx{t}", name=f"vx{t}") for t in range(TK)]

            def project_v(t):
                ps = ps_proj.tile([128, E], f32, tag="mm512", name="vps")
                for k in range(EC):
                    nc.tensor.matmul(
                        ps[:],
                        xhatT[k][:, t * 128:(t + 1) * 128],
                        wv_sb[k][:],
                        start=(k == 0),
                        stop=(k == EC - 1),
                    )
                vv = vext[t].rearrange("p (h c) -> p h c", c=VW)
                nc.vector.tensor_add(
                    vv[:, :, 0:D],
                    ps[:].rearrange("p (h c) -> p h c", c=D),
                    bv_rep[:].rearrange("p (h c) -> p h c", c=D),
                )
                nc.gpsimd.memset(vv[:, :, D:VW], 1.0)

            for dc in range(EC):
                for h2 in range(SQ // 512):
                    ps = ps_proj.tile([128, 512], f32, tag="mm512")
                    for k in range(EC):
                        nc.tensor.matmul(
                            ps[:],
                            wq_sb[k][:, dc * 128:(dc + 1) * 128],
                            xhatT[k][:, h2 * 512:(h2 + 1) * 512],
                            start=(k == 0),
                            stop=(k == EC - 1),
                        )
                    nc.vector.tensor_scalar(
                        out=qT[dc][:, h2 * 512:(h2 + 1) * 512],
                        in0=ps[:],
                        scalar1=bqT[:, dc:dc + 1],
                        scalar2=None,
                        op0=ALU.add,
                    )
                for h2 in range(S // 512):
                    ps = ps_proj.tile([128, 512], f32, tag="mm512")
                    for k in range(EC):
                        nc.tensor.matmul(
                            ps[:],
                            wk_sb[k][:, dc * 128:(dc + 1) * 128],
                            xhatT[k][:, h2 * 512:(h2 + 1) * 512],
                            start=(k == 0),
                            stop=(k == EC - 1),
                        )
                    nc.vector.tensor_scalar(
                        out=kT[dc][:, h2 * 512:(h2 + 1) * 512],
                        in0=ps[:],
                        scalar1=bkT[:, dc:dc + 1],
                        scalar2=None,
                        op0=ALU.add,
                    )
                if dc == 0:
                    for t in range(TK):
                        project_v(t)

            projsc.close()
            ps_ctx = attn.enter_context(
                tc.tile_pool(name="ps_ctx", bufs=2, space="PSUM")
            )

            # ---- phase C: attention, head pairs ----
            att_sb = [resA.tile([128, E], f32, tag=f"att{t}", name=f"att{t}") for t in range(TQ)]
            ptodd = [resA.tile([128, SQ], bf16, tag=f"pto{kc}", name=f"pto{kc}") for kc in range(TK)]

            def head_epilogue(h, ctx_ps):
                ctxs = tempA.tile([VW, SQ], f32, tag="ctxs")
                nc.vector.tensor_copy(ctxs[:], ctx_ps[:])
                for qc in range(TQ):
                    tp = ps_st.tile([128, VW], f32, tag="st", name="tp")
                    nc.tensor.transpose(
                        tp[:], ctxs[:, qc * 128:(qc + 1) * 128], ident32[0:VW, 0:VW]
                    )
                    rec = stats.tile([128, 1], f32, tag="zrec")
                    nc.vector.reciprocal(rec[:], tp[:, D:VW])
                    nc.vector.tensor_scalar(
                        out=att_sb[qc][:, h * D:(h + 1) * D],
                        in0=tp[:, 0:D],
                        scalar1=rec[:],
                        scalar2=None,
                        op0=ALU.mult,
                    )

            for hp in range(H // 2):
                h0, h1 = 2 * hp, 2 * hp + 1
                ctx0 = ps_ctx.tile([VW, SQ], f32, tag="ctx", name="ctx0")
                for kc in range(TK):
                    pt0 = tempA.tile([128, SQ], bf16, tag="pte", name="pte")
                    pts = {h0: pt0, h1: ptodd[kc]}
                    for half in range(SQ // 512):
                        for h in (h0, h1):
                            dc, row = h // 2, (h % 2) * D
                            st = ps_st.tile([128, 512], f32, tag="st", name="st")
                            nc.tensor.matmul(
                                st[:],
                                kT[dc][row:row + D, kc * 128:(kc + 1) * 128],
                                qT[dc][row:row + D, half * 512:(half + 1) * 512],
                                start=True,
                                stop=True,
                            )
                            nc.scalar.activation(
                                pts[h][:, half * 512:(half + 1) * 512], st[:], AF.Exp
                            )
                        nc.tensor.matmul(
                            ctx0[:, half * 512:(half + 1) * 512],
                            vext[kc][:, h0 * VW:(h0 + 1) * VW],
                            pt0[:, half * 512:(half + 1) * 512],
                            start=(kc == 0),
                            stop=(kc == TK - 1),
                        )
                head_epilogue(h0, ctx0)
                ctx1 = ps_ctx.tile([VW, SQ], f32, tag="ctx", name="ctx1")
                for kc in range(TK):
                    for half in range(SQ // 512):
                        nc.tensor.matmul(
                            ctx1[:, half * 512:(half + 1) * 512],
                            vext[kc][:, h1 * VW:(h1 + 1) * VW],
                            ptodd[kc][:, half * 512:(half + 1) * 512],
                            start=(kc == 0),
                            stop=(kc == TK - 1),
                        )
                head_epilogue(h1, ctx1)

            # ---- phase D: residual ----
            for qc in range(TQ):
                nc.gpsimd.tensor_add(x1[qc][:], xq[qc][:], att_sb[qc][:])

        # ---- phase E/F/G: LN2 + FFN ----
        with ExitStack() as ffn:
            resB = ffn.enter_context(tc.tile_pool(name="resB", bufs=1))
            tempB = ffn.enter_context(tc.tile_pool(name="tempB", bufs=3))
            ps_h1 = ffn.enter_context(tc.tile_pool(name="ps_h1", bufs=2, space="PSUM"))
            ps_f2 = ffn.enter_context(tc.tile_pool(name="ps_f2", bufs=2, space="PSUM"))

            xh2T = [resB.tile([128, SQ], f32r, tag=f"x2T{k}", name=f"x2T{k}") for k in range(EC)]
            for qc in range(TQ):
                xh2 = tempB.tile([128, E], f32, tag="xh2")
                layernorm_to(xh2[:], x1[qc][:])
                for k in range(EC):
                    tp = ps_f2.tile([128, 128], f32, tag="t32")
                    nc.tensor.transpose(
                        tp[:], xh2[:, k * 128:(k + 1) * 128], ident32[:]
                    )
                    nc.scalar.copy(xh2T[k][:, qc * 128:(qc + 1) * 128], tp[:])

            w1_sb = [resB.tile([128, F], f32r, tag=f"w1{k}", name=f"w1{k}") for k in range(EC)]
            for k in range(EC):
                nc.sync.dma_start(w1_sb[k][:], w1_d[k])
            b1T = resB.tile([128, FC], f32, tag="b1T")
            nc.sync.dma_start(b1T[:], b1T_d[:])
            w2_sb = [resB.tile([128, E], f16, tag=f"w2{c}", name=f"w2{c}") for c in range(FC)]
            for c in range(FC):
                nc.sync.dma_start(w2_sb[c][:], w2_d[c])

            spkT = [resB.tile([128, SQ], f16, tag=f"spk{c}", name=f"spk{c}") for c in range(FC)]
            for half in range(SQ // 512):
                for c in range(FC):
                    ps = ps_h1.tile([128, 512], f32, tag="h1")
                    for k in range(EC):
                        nc.tensor.matmul(
                            ps[:],
                            w1_sb[k][:, c * 128:(c + 1) * 128],
                            xh2T[k][:, half * 512:(half + 1) * 512],
                            start=(k == 0),
                            stop=(k == EC - 1),
                        )
                    nc.vector.tensor_scalar(
                        out=spkT[c][:, half * 512:(half + 1) * 512],
                        in0=ps[:],
                        scalar1=b1T[:, c:c + 1],
                        scalar2=2.0,
                        op0=ALU.add,
                        op1=ALU.is_ge,
                    )
                for qc in range(half * (TQ // 2), (half + 1) * (TQ // 2)):
                    ps = ps_f2.tile([128, E], f32, tag="mm512")
                    for c in range(FC):
                        nc.tensor.matmul(
                            ps[:],
                            spkT[c][:, qc * 128:(qc + 1) * 128],
                            w2_sb[c][:],
                            start=(c == 0),
                            stop=False,
                        )
                    nc.tensor.matmul(ps[:], ones1[:], b2_sb[:], start=False, stop=True)
                    ot = outp.tile([128, E], f32, tag="ot")
                    nc.vector.tensor_add(ot[:], x1[qc][:], ps[:])
                    nc.sync.dma_start(out_d[qc * 128:(qc + 1) * 128, :], ot[:])

    if split:
        split_multiwait(nc)
    return nc


_NC = None


def _get_nc():
    global _NC
    if _NC is None:
        _NC = build_nc()
    return _NC


# --------------------------------------------------------------------------
# Host wrapper
# --------------------------------------------------------------------------

def _prep_weights(inputs):
    f = lambda k: np.asarray(inputs[k], np.float32)
    g1, be1 = f("g1"), f("be1")
    g2, be2 = f("g2"), f("be2")
    wq, wk, wv = f("wq"), f("wk"), f("wv")
    bq, bk, bv = f("bq"), f("bk"), f("bv")
    w1, b1 = f("w1"), f("b1")
    w2, b2 = f("w2"), f("b2")

    wq_e = wq * g1[:, None]
    wk_e = wk * g1[:, None]
    wv_e = wv * g1[:, None]
    bq_e = bq + be1 @ wq
    bk_e = bk + be1 @ wk
    bv_e = bv + be1 @ wv
    w1_e = w1 * g2[:, None]
    b1_e = b1 + be2 @ w1

    return {
        "wq": wq_e.reshape(EC, 128, E).astype(np.float16),
        "wk": wk_e.reshape(EC, 128, E).astype(np.float16),
        "wv": wv_e.reshape(EC, 128, E).astype(np.float16),
        "bqT": np.ascontiguousarray(bq_e.reshape(EC, 128).T),
        "bkT": np.ascontiguousarray(bk_e.reshape(EC, 128).T),
        "bv": bv_e,
        "w1": np.ascontiguousarray(w1_e.reshape(EC, 128, F)),
        "b1T": np.ascontiguousarray(b1_e.reshape(FC, 128).T),
        "w2": np.ascontiguousarray(w2.reshape(FC, 128, E)).astype(np.float16),
        "b2": b2.reshape(1, E).astype(np.float16),
    }


def _run(inputs, **spmd_kwargs):
    x = np.asarray(inputs["x"], np.float32)
    w = _prep_weights(inputs)
    in_maps = []
    for c in range(N_CORES):
        b, h = c // 2, c % 2
        xq = x[b, h * SQ:(h + 1) * SQ]
        xo = x[b, (1 - h) * SQ:(2 - h) * SQ]
        m = dict(w)
        m["xin"] = np.ascontiguousarray(np.concatenate([xq, xo], axis=0))
        in_maps.append(m)
    res = run_bass_kernel_spmd(_get_nc(), in_maps, list(range(N_CORES)), **spmd_kwargs)
    out = np.empty((M, S, E), np.float32)
    for c in range(N_CORES):
        b, h = c // 2, c % 2
        out[b, h * SQ:(h + 1) * SQ] = res.results[c]["out"]
    return out, res


def kernel(**inputs):
    try:
        out, _ = _run(inputs)
    except Exception:
        # transient device hiccups (NRT exec-unit resets) recover on retry
        out, _ = _run(inputs)
    return out


# revision 39
# speedup vs baseline: 1.2534x; 1.0116x over previous
"""Trainium2 Bass kernel for nn_Encoder (pre-norm attention + spiking FFN).

Sharding: 8 cores = 4 batches x 2 sequence halves, pure data parallel, no
collectives.  Each core receives the full 2048-token batch row with its own
query half permuted to the front (softmax over keys is permutation
invariant), computes attention for its 1024 query tokens against all 2048
keys, plus the FFN for those tokens, and returns a [1024, 512] slice.

LayerNorm affine params and linear biases are folded on the host:
  n = xhat*g + be  =>  n @ W + b == xhat @ (g[:,None]*W) + (be@W + b)
so the device only computes plain (x-mu)*rstd layernorms.

Math per core (m-batch row, q = first 1024 tokens of xin):
  xhat1 = LN(xin)                         (all 2048 tokens)
  qT/kT = (wq'/wk')^T xhat1^T + b^T       (f16, transposed layout)
  v     = xhat1 @ wv' + bv'               (f16, natural layout, +ones col)
  S^T   = kT_h^T q_h per head             (PSUM f32)
  P^T   = exp(S^T)                        (f16; no max subtraction - scores
                                           are O(10) so f32 exp is safe)
  ctx^T = [V_h|1]^T P^T  accumulated over key chunks  -> [65, 1024]
  att   = transpose(ctx^T) rows scaled by 1/Z (Z = ones-col sums)
  x1    = xq + att
  h1^T  = w1'^T LN(x1)^T + b1'            (f32r for spike-threshold accuracy)
  spk^T = (h1^T >= 2.0)                   (f16, exact 0/1)
  out   = x1 + spk @ w2 + b2              (b2 via K=1 ones matmul)
"""

import sys
from contextlib import ExitStack

sys.path.insert(0, "/opt/trn_rl_repo")

import numpy as np

import concourse.bass as bass
import concourse.tile as tile
from concourse import mybir
from concourse.bass_utils import run_bass_kernel_spmd
from concourse.masks import make_identity
from concourse.vector_clock import ScopedClock, VectorClock

f32 = mybir.dt.float32
f32r = mybir.dt.float32r
f16 = mybir.dt.float16
bf16 = mybir.dt.bfloat16
AF = mybir.ActivationFunctionType
ALU = mybir.AluOpType

M, S, E, H, D, F = 4, 2048, 512, 8, 64, 2048
SQ = S // 2              # query tokens per core
N_CORES = 8
EPS = 1e-5
EC = E // 128            # 4 embed chunks
FC = F // 128            # 16 ffn chunks
TK = S // 128            # 16 key-token tiles
TQ = SQ // 128           # 8 query-token tiles
VW = D + 1               # per-head Vext width (64 v cols + ones col)


# --------------------------------------------------------------------------
# Tile framework patches for this toolchain: walrus rejects >1 sem-wait per
# instruction, so (a) the TileContext exit drain is replaced with a chain of
# single-wait SP nops, and (b) a post-pass splits any remaining multi-wait
# instruction into same-engine single-wait NoOps placed immediately before it
# (engines execute in order, so the wait point is unchanged).
# --------------------------------------------------------------------------

def _split_drain_and_barrier(self, tick_clock, wait_clock):
    g = tick_clock.global_clock
    n = len(g)
    for p in range(n):
        if g[p] > 0:
            vec = [g[p] if i == p else 0 for i in range(n)]
            nop = self.nc.sync.nop(nofuse=True, hint="split_drain")
            wait_clock.add_sem_waits(nop.ins, ScopedClock({None: VectorClock(vec)}))
    self.nc.sync.drain()
    self.nc.all_engine_barrier()
    assert self.sems is not None
    popped = self.nc._tile_sem_poison_stack.pop()
    assert popped is self._sem_poison
    self.nc.clear_and_free_semaphores(list(self.sems.allocated().values()))
    self.nc.all_engine_barrier()


tile.TileContext._drain_and_barrier = _split_drain_and_barrier


def split_multiwait(nc, limit=1):
    n_split = 0
    for fn in nc.m.functions:
        for bb in fn.blocks:
            il = bb.instructions
            out = []
            for inst in il:
                si = getattr(inst, "sync_info", None)
                waits = list(si.on_wait) if si is not None and si.on_wait else []
                if len(waits) > limit:
                    keep = waits[-limit:]
                    extra = waits[:-limit]
                    for j, w in enumerate(extra):
                        nop = mybir.InstNoOp(name=f"{inst.name}-wsplit{j}")
                        nop.engine = inst.engine
                        nop.sync_info = mybir.SyncInfo(on_wait=[w], on_update=[])
                        out.append(nop)
                        n_split += 1
                    inst.sync_info = mybir.SyncInfo(
                        on_wait=keep, on_update=list(si.on_update)
                    )
                out.append(inst)
            if len(out) != len(il):
                il[:] = out
    return n_split


# --------------------------------------------------------------------------
# Device program
# --------------------------------------------------------------------------

def build_nc(split=True):
    nc = bass.Bass()

    xin = nc.declare_dram_parameter("xin", [S, E], f32, isOutput=False)
    wq_d = nc.declare_dram_parameter("wq", [EC, 128, E], f16, isOutput=False)
    wk_d = nc.declare_dram_parameter("wk", [EC, 128, E], f16, isOutput=False)
    wv_d = nc.declare_dram_parameter("wv", [EC, 128, E], f16, isOutput=False)
    bqT_d = nc.declare_dram_parameter("bqT", [128, EC], f32, isOutput=False)
    bkT_d = nc.declare_dram_parameter("bkT", [128, EC], f32, isOutput=False)
    bv_d = nc.declare_dram_parameter("bv", [E], f32, isOutput=False)
    w1_d = nc.declare_dram_parameter("w1", [EC, 128, F], f32r, isOutput=False)
    b1T_d = nc.declare_dram_parameter("b1T", [128, FC], f32, isOutput=False)
    w2_d = nc.declare_dram_parameter("w2", [FC, 128, E], f16, isOutput=False)
    b2_d = nc.declare_dram_parameter("b2", [1, E], f16, isOutput=False)
    out_d = nc.declare_dram_parameter("out", [SQ, E], f32, isOutput=True)

    with tile.TileContext(nc) as tc, ExitStack() as top:
        common = top.enter_context(tc.tile_pool(name="common", bufs=1))
        stats = top.enter_context(tc.tile_pool(name="stats", bufs=4))
        outp = top.enter_context(tc.tile_pool(name="outp", bufs=3))

        ident16 = common.tile([128, 128], f16, tag="ident16")
        make_identity(nc, ident16[:])
        ident32 = common.tile([128, 128], f32, tag="ident32")
        make_identity(nc, ident32[:])
        ones1 = common.tile([1, 128], f16, tag="ones1")
        nc.vector.memset(ones1[:], 1.0)
        b2_sb = common.tile([1, E], f16, tag="b2")
        nc.sync.dma_start(b2_sb[:], b2_d[:])
        bv_rep = common.tile([128, E], f32, tag="bvrep")
        bv_ap = bv_d[:]
        nc.gpsimd.dma_start(
            out=bv_rep[:],
            in_=bass.AP(tensor=bv_ap.tensor, offset=bv_ap.offset,
                        ap=[[0, 128]] + list(bv_ap.ap)),
        )
        x1 = [common.tile([128, E], f32, tag=f"x1_{t}", name=f"x1_{t}") for t in range(TQ)]
        eps_sb = common.tile([128, 1], f32, tag="eps")
        nc.vector.memset(eps_sb[:], EPS)

        def layernorm_to(dst_f16_or_f32, src_ap):
            st6 = stats.tile([128, 6], f32, tag="bn6")
            nc.vector.bn_stats(st6[:], src_ap)
            mv = stats.tile([128, 2], f32, tag="mv")
            nc.vector.bn_aggr(mv[:], st6[:])
            std = stats.tile([128, 1], f32, tag="std")
            nc.scalar.activation(std[:], mv[:, 1:2], AF.Sqrt, bias=eps_sb[:])
            rstd = stats.tile([128, 1], f32, tag="rstd")
            nc.vector.reciprocal(rstd[:], std[:])
            nc.vector.tensor_scalar(
                out=dst_f16_or_f32,
                in0=src_ap,
                scalar1=mv[:, 0:1],
                scalar2=rstd[:],
                op0=ALU.subtract,
                op1=ALU.mult,
            )

        with ExitStack() as attn:
            resA = attn.enter_context(tc.tile_pool(name="resA", bufs=1))
            tempA = attn.enter_context(tc.tile_pool(name="tempA", bufs=3))
            ps_st = attn.enter_context(tc.tile_pool(name="ps_st", bufs=4, space="PSUM"))
            projsc = ExitStack()
            ps_proj = projsc.enter_context(
                tc.tile_pool(name="ps_proj", bufs=2, space="PSUM")
            )

            # ---- phase A: load x, LN1, transpose to xhatT (f16) ----
            xq = [resA.tile([128, E], f32, tag=f"xq{t}", name=f"xq{t}") for t in range(TQ)]
            xhatT = [resA.tile([128, S], f16, tag=f"xhT{k}", name=f"xhT{k}") for k in range(EC)]
            for t in range(TK):
                if t < TQ:
                    xt = xq[t]
                else:
                    xt = tempA.tile([128, E], f32, tag="xkv")
                nc.sync.dma_start(xt[:], xin[t * 128:(t + 1) * 128, :])
                xh = tempA.tile([128, E], f16, tag="xh1")
                layernorm_to(xh[:], xt[:])
                for k in range(EC):
                    tp = ps_proj.tile([128, 128], f16, tag="t16")
                    nc.tensor.transpose(tp[:], xh[:, k * 128:(k + 1) * 128], ident16[:])
                    nc.vector.tensor_copy(xhatT[k][:, t * 128:(t + 1) * 128], tp[:])

            # ---- phase B: projections ----
            wq_sb = [resA.tile([128, E], f16, tag=f"wq{k}", name=f"wq{k}") for k in range(EC)]
            wk_sb = [resA.tile([128, E], f16, tag=f"wk{k}", name=f"wk{k}") for k in range(EC)]
            wv_sb = [resA.tile([128, E], f16, tag=f"wv{k}", name=f"wv{k}") for k in range(EC)]
            for k in range(EC):
                nc.sync.dma_start(wq_sb[k][:], wq_d[k])
                nc.sync.dma_start(wk_sb[k][:], wk_d[k])
                nc.sync.dma_start(wv_sb[k][:], wv_d[k])
            bqT = resA.tile([128, EC], f32, tag="bqT")
            bkT = resA.tile([128, EC], f32, tag="bkT")
            nc.sync.dma_start(bqT[:], bqT_d[:])
            nc.sync.dma_start(bkT[:], bkT_d[:])

            qT = [resA.tile([128, SQ], f16, tag=f"qT{k}", name=f"qT{k}") for k in range(EC)]
            kT = [resA.tile([128, S], f16, tag=f"kT{k}", name=f"kT{k}") for k in range(EC)]
            vext = [resA.tile([128, H * VW], bf16, tag=f"vx{t}", name=f"vx{t}") for t in range(TK)]

            def project_v(t):
                ps = ps_proj.tile([128, E], f32, tag="mm512", name="vps")
                for k in range(EC):
                    nc.tensor.matmul(
                        ps[:],
                        xhatT[k][:, t * 128:(t + 1) * 128],
                        wv_sb[k][:],
                        start=(k == 0),
                        stop=(k == EC - 1),
                    )
                vv = vext[t].rearrange("p (h c) -> p h c", c=VW)
                nc.vector.tensor_add(
                    vv[:, :, 0:D],
                    ps[:].rearrange("p (h c) -> p h c", c=D),
                    bv_rep[:].rearrange("p (h c) -> p h c", c=D),
                )
                nc.gpsimd.memset(vv[:, :, D:VW], 1.0)

            for dc in range(EC):
                for h2 in range(SQ // 512):
                    ps = ps_proj.tile([128, 512], f32, tag="mm512")
                    for k in range(EC):
                        nc.tensor.matmul(
                            ps[:],
                            wq_sb[k][:, dc * 128:(dc + 1) * 128],
                            xhatT[k][:, h2 * 512:(h2 + 1) * 512],
                            start=(k == 0),
                            stop=(k == EC - 1),
                        )
                    nc.vector.tensor_scalar(
                        out=qT[dc][:, h2 * 512:(h2 + 1) * 512],
                        in0=ps[:],
                        scalar1=bqT[:, dc:dc + 1],
                        scalar2=None,
                        op0=ALU.add,
                    )
                for h2 in range(S // 512):
                    ps = ps_proj.tile([128, 512], f32, tag="mm512")
                    for k in range(EC):
                        nc.tensor.matmul(
                            ps[:],
                            wk_sb[k][:, dc * 128:(dc + 1) * 128],
                            xhatT[k][:, h2 * 512:(h2 + 1) * 512],
                            start=(k == 0),
                            stop=(k == EC - 1),
                        )
                    nc.vector.tensor_scalar(
                        out=kT[dc][:, h2 * 512:(h2 + 1) * 512],
                        in0=ps[:],
                        scalar1=bkT[:, dc:dc + 1],
                        scalar2=None,
                        op0=ALU.add,
                    )
                if dc == 0:
                    for t in range(TK):
                        project_v(t)

            projsc.close()
            ps_ctx = attn.enter_context(
                tc.tile_pool(name="ps_ctx", bufs=2, space="PSUM")
            )

            # ---- phase C: attention, head pairs ----
            att_sb = [resA.tile([128, E], f32, tag=f"att{t}", name=f"att{t}") for t in range(TQ)]
            ptodd = [resA.tile([128, SQ], bf16, tag=f"pto{kc}", name=f"pto{kc}") for kc in range(TK)]

            def head_epilogue(h, ctx_ps):
                ctxs = tempA.tile([VW, SQ], f32, tag="ctxs")
                nc.vector.tensor_copy(ctxs[:], ctx_ps[:])
                for qc in range(TQ):
                    tp = ps_st.tile([128, VW], f32, tag="st", name="tp")
                    nc.tensor.transpose(
                        tp[:], ctxs[:, qc * 128:(qc + 1) * 128], ident32[0:VW, 0:VW]
                    )
                    rec = stats.tile([128, 1], f32, tag="zrec")
                    nc.vector.reciprocal(rec[:], tp[:, D:VW])
                    nc.vector.tensor_scalar(
                        out=att_sb[qc][:, h * D:(h + 1) * D],
                        in0=tp[:, 0:D],
                        scalar1=rec[:],
                        scalar2=None,
                        op0=ALU.mult,
                    )

            for hp in range(H // 2):
                h0, h1 = 2 * hp, 2 * hp + 1
                ctx0 = ps_ctx.tile([VW, SQ], f32, tag="ctx", name="ctx0")
                for kc in range(TK):
                    pt0 = tempA.tile([128, SQ], bf16, tag="pte", name="pte")
                    pts = {h0: pt0, h1: ptodd[kc]}
                    for half in range(SQ // 512):
                        for h in (h0, h1):
                            dc, row = h // 2, (h % 2) * D
                            st = ps_st.tile([128, 512], f32, tag="st", name="st")
                            nc.tensor.matmul(
                                st[:],
                                kT[dc][row:row + D, kc * 128:(kc + 1) * 128],
                                qT[dc][row:row + D, half * 512:(half + 1) * 512],
                                start=True,
                                stop=True,
                            )
                            nc.scalar.activation(
                                pts[h][:, half * 512:(half + 1) * 512], st[:], AF.Exp
                            )
                        nc.tensor.matmul(
                            ctx0[:, half * 512:(half + 1) * 512],
                            vext[kc][:, h0 * VW:(h0 + 1) * VW],
                            pt0[:, half * 512:(half + 1) * 512],
                            start=(kc == 0),
                            stop=(kc == TK - 1),
                        )
                head_epilogue(h0, ctx0)
                ctx1 = ps_ctx.tile([VW, SQ], f32, tag="ctx", name="ctx1")
                for half in range(SQ // 512):
                    for kc in range(TK):
                        nc.tensor.matmul(
                            ctx1[:, half * 512:(half + 1) * 512],
                            vext[kc][:, h1 * VW:(h1 + 1) * VW],
                            ptodd[kc][:, half * 512:(half + 1) * 512],
                            start=(kc == 0),
                            stop=(kc == TK - 1),
                        )
                head_epilogue(h1, ctx1)

            # ---- phase D: residual ----
            for qc in range(TQ):
                nc.gpsimd.tensor_add(x1[qc][:], xq[qc][:], att_sb[qc][:])

        # ---- phase E/F/G: LN2 + FFN ----
        with ExitStack() as ffn:
            resB = ffn.enter_context(tc.tile_pool(name="resB", bufs=1))
            tempB = ffn.enter_context(tc.tile_pool(name="tempB", bufs=3))
            ps_h1 = ffn.enter_context(tc.tile_pool(name="ps_h1", bufs=2, space="PSUM"))
            ps_f2 = ffn.enter_context(tc.tile_pool(name="ps_f2", bufs=2, space="PSUM"))

            xh2T = [resB.tile([128, SQ], f32r, tag=f"x2T{k}", name=f"x2T{k}") for k in range(EC)]
            for qc in range(TQ):
                xh2 = tempB.tile([128, E], f32, tag="xh2")
                layernorm_to(xh2[:], x1[qc][:])
                for k in range(EC):
                    tp = ps_f2.tile([128, 128], f32, tag="t32")
                    nc.tensor.transpose(
                        tp[:], xh2[:, k * 128:(k + 1) * 128], ident32[:]
                    )
                    nc.scalar.copy(xh2T[k][:, qc * 128:(qc + 1) * 128], tp[:])

            w1_sb = [resB.tile([128, F], f32r, tag=f"w1{k}", name=f"w1{k}") for k in range(EC)]
            for k in range(EC):
                nc.sync.dma_start(w1_sb[k][:], w1_d[k])
            b1T = resB.tile([128, FC], f32, tag="b1T")
            nc.sync.dma_start(b1T[:], b1T_d[:])
            w2_sb = [resB.tile([128, E], f16, tag=f"w2{c}", name=f"w2{c}") for c in range(FC)]
            for c in range(FC):
                nc.sync.dma_start(w2_sb[c][:], w2_d[c])

            spkT = [resB.tile([128, SQ], f16, tag=f"spk{c}", name=f"spk{c}") for c in range(FC)]
            for half in range(SQ // 512):
                for c in range(FC):
                    ps = ps_h1.tile([128, 512], f32, tag="h1")
                    for k in range(EC):
                        nc.tensor.matmul(
                            ps[:],
                            w1_sb[k][:, c * 128:(c + 1) * 128],
                            xh2T[k][:, half * 512:(half + 1) * 512],
                            start=(k == 0),
                            stop=(k == EC - 1),
                        )
                    nc.vector.tensor_scalar(
                        out=spkT[c][:, half * 512:(half + 1) * 512],
                        in0=ps[:],
                        scalar1=b1T[:, c:c + 1],
                        scalar2=2.0,
                        op0=ALU.add,
                        op1=ALU.is_ge,
                    )
                for qc in range(half * (TQ // 2), (half + 1) * (TQ // 2)):
                    ps = ps_f2.tile([128, E], f32, tag="mm512")
                    for c in range(FC):
                        nc.tensor.matmul(
                            ps[:],
                            spkT[c][:, qc * 128:(qc + 1) * 128],
                            w2_sb[c][:],
                            start=(c == 0),
                            stop=False,
                        )
                    nc.tensor.matmul(ps[:], ones1[:], b2_sb[:], start=False, stop=True)
                    ot = outp.tile([128, E], f32, tag="ot")
                    nc.vector.tensor_add(ot[:], x1[qc][:], ps[:])
                    nc.sync.dma_start(out_d[qc * 128:(qc + 1) * 128, :], ot[:])

    if split:
        split_multiwait(nc)
    return nc


_NC = None


def _get_nc():
    global _NC
    if _NC is None:
        _NC = build_nc()
    return _NC


# --------------------------------------------------------------------------
# Host wrapper
# --------------------------------------------------------------------------

def _prep_weights(inputs):
    f = lambda k: np.asarray(inputs[k], np.float32)
    g1, be1 = f("g1"), f("be1")
    g2, be2 = f("g2"), f("be2")
    wq, wk, wv = f("wq"), f("wk"), f("wv")
    bq, bk, bv = f("bq"), f("bk"), f("bv")
    w1, b1 = f("w1"), f("b1")
    w2, b2 = f("w2"), f("b2")

    wq_e = wq * g1[:, None]
    wk_e = wk * g1[:, None]
    wv_e = wv * g1[:, None]
    bq_e = bq + be1 @ wq
    bk_e = bk + be1 @ wk
    bv_e = bv + be1 @ wv
    w1_e = w1 * g2[:, None]
    b1_e = b1 + be2 @ w1

    return {
        "wq": wq_e.reshape(EC, 128, E).astype(np.float16),
        "wk": wk_e.reshape(EC, 128, E).astype(np.float16),
        "wv": wv_e.reshape(EC, 128, E).astype(np.float16),
        "bqT": np.ascontiguousarray(bq_e.reshape(EC, 128).T),
        "bkT": np.ascontiguousarray(bk_e.reshape(EC, 128).T),
        "bv": bv_e,
        "w1": np.ascontiguousarray(w1_e.reshape(EC, 128, F)),
        "b1T": np.ascontiguousarray(b1_e.reshape(FC, 128).T),
        "w2": np.ascontiguousarray(w2.reshape(FC, 128, E)).astype(np.float16),
        "b2": b2.reshape(1, E).astype(np.float16),
    }


def _run(inputs, **spmd_kwargs):
    x = np.asarray(inputs["x"], np.float32)
    w = _prep_weights(inputs)
    in_maps = []
    for c in range(N_CORES):
        b, h = c // 2, c % 2
        xq = x[b, h * SQ:(h + 1) * SQ]
        xo = x[b, (1 - h) * SQ:(2 - h) * SQ]
        m = dict(w)
        m["xin"] = np.ascontiguousarray(np.concatenate([xq, xo], axis=0))
        in_maps.append(m)
    res = run_bass_kernel_spmd(_get_nc(), in_maps, list(range(N_CORES)), **spmd_kwargs)
    out = np.empty((M, S, E), np.float32)
    for c in range(N_CORES):
        b, h = c // 2, c % 2
        out[b, h * SQ:(h + 1) * SQ] = res.results[c]["out"]
    return out, res


def kernel(**inputs):
    try:
        out, _ = _run(inputs)
    except Exception:
        # transient device hiccups (NRT exec-unit resets) recover on retry
        out, _ = _run(inputs)
    return out
